# revision 1
# baseline (speedup 1.0000x reference)
"""Causal self-attention (B=2, S=2048, D=2048, H=16) on 8 TRN2 NeuronCores.

Sharding: 2 batches x 4 head-groups.  Core c handles batch c//4 and heads
[4*(c%4) .. 4*(c%4)+3].  Per core:
  phase 1: qT/kT (transposed) + v projections from host-pre-transposed xT
  phase 2: causal attention per (q-block, head) with exp-softmax (no max
           subtraction -- logits are O(8) here), denominators via ones-matmul
  AllGather (groups of 4) of normalized yT (bf16) per q-block chunk
  phase 3: out projection of the core's 512 output columns + bias
Host side: slice/convert inputs (bf16), assemble the 8 [2048,512] results.

Compute is bf16 with fp32 PSUM accumulation; measured l2 rel err vs the fp32
reference is ~5.5e-3.
"""

import numpy as np
import ml_dtypes

B, S, D = 2, 2048, 2048
H, HD = 16, 128
HLOC = 4           # heads per core
CW = HLOC * HD     # 512: per-core q/k/v width and out-column width
QB = 4             # q blocks of 512
DT = 16            # d tiles of 128
TB = 4             # token blocks of 512
SCALE = 1.0 / float(np.sqrt(HD))
GROUPS = [[0, 1, 2, 3], [4, 5, 6, 7]]

_cache = {}


def _build():
    import concourse.tile as tile
    import concourse.mybir as mybir
    from concourse import bacc
    from concourse.masks import make_identity

    BF = mybir.dt.bfloat16
    F32 = mybir.dt.float32

    nc = bacc.Bacc("TRN2", target_bir_lowering=False, debug=False, num_devices=8)

    # Inputs (per-core shards, host-prepared)
    xT = nc.dram_tensor("xT", [D, S], BF, kind="ExternalInput")            # x[batch].T
    wqk = nc.dram_tensor("wqk", [DT, 8, 128, 128], BF, kind="ExternalInput")  # (dt, ct, d, col)
    wv = nc.dram_tensor("wv", [DT, 128, CW], BF, kind="ExternalInput")     # (dt, d, vcol)
    bqk = nc.dram_tensor("bqk", [8, 128, 1], F32, kind="ExternalInput")
    bv = nc.dram_tensor("bv", [1, CW], F32, kind="ExternalInput")
    wout = nc.dram_tensor("wout", [D, CW], BF, kind="ExternalInput")
    bout = nc.dram_tensor("bout", [1, CW], F32, kind="ExternalInput")
    out = nc.dram_tensor("out", [S, CW], F32, kind="ExternalOutput")

    ag_in = [nc.dram_tensor(f"ag_in{qb}", [CW, 512], BF, kind="Internal")
             for qb in range(QB)]
    ag_out = [nc.dram_tensor(f"ag_out{qb}", [D, 512], BF, kind="Internal")
              for qb in range(QB)]

    with tile.TileContext(nc) as tc:
        with (
            tc.tile_pool(name="const", bufs=1) as constp,
            tc.tile_pool(name="pers", bufs=1) as pers,
            tc.tile_pool(name="work", bufs=2) as work,
            tc.tile_pool(name="psum", bufs=2, space="PSUM") as psum,
        ):
            # ---- constants ----
            ones = constp.tile([128, 1], BF, name="ones")
            nc.gpsimd.memset(ones[:], 1.0)

            masks = []
            for p in range(4):
                m = constp.tile([128, 512], BF, name=f"mask{p}", tag=f"mask{p}")
                nc.gpsimd.memset(m[:], 1.0)
                # keep (1.0) where qq >= kk + 128*p, else 0.0
                nc.gpsimd.affine_select(
                    out=m[:], in_=m[:],
                    compare_op=mybir.AluOpType.is_ge, fill=0.0,
                    base=-128 * p, channel_multiplier=-1, pattern=[[1, 512]],
                )
                masks.append(m)

            bout_sb = constp.tile([1, CW], F32, name="bout_sb")
            nc.sync.dma_start(bout_sb[:], bout[:])
            bias_bc = constp.tile([128, CW], F32, name="bias_bc")
            nc.gpsimd.partition_broadcast(bias_bc[:], bout_sb[:], channels=128)

            bv_sb = constp.tile([1, CW], F32, name="bv_sb")
            nc.sync.dma_start(bv_sb[:], bv[:])
            vbias_bc = constp.tile([128, CW], F32, name="vbias_bc")
            nc.gpsimd.partition_broadcast(vbias_bc[:], bv_sb[:], channels=128)

            bqk_sb = []
            for ct in range(8):
                t = constp.tile([128, 1], F32, name=f"bqk{ct}", tag=f"bqk{ct}")
                nc.sync.dma_start(t[:], bqk[ct])
                bqk_sb.append(t)

            # ---- persistent SBUF tensors ----
            qT = [pers.tile([128, S], BF, name=f"qT{h}", tag=f"qT{h}")
                  for h in range(HLOC)]
            kT = [pers.tile([128, S], BF, name=f"kT{h}", tag=f"kT{h}")
                  for h in range(HLOC)]
            vt = [pers.tile([128, CW], BF, name=f"v{t}", tag=f"v{t}")
                  for t in range(16)]
            yT = [pers.tile([128, S], BF, name=f"yT{h}", tag=f"yT{h}")
                  for h in range(HLOC)]

            # ---- load xT (64 tiles [128 d, 512 tok], bufs shared with ygT) ----
            xt_tiles = {}
            for dt in range(DT):
                for tb in range(TB):
                    t = work.tile([128, 512], BF, name=f"xt_{dt}_{tb}",
                                  tag="xT", bufs=80)
                    nc.sync.dma_start(
                        t[:], xT[dt * 128:(dt + 1) * 128, tb * 512:(tb + 1) * 512])
                    xt_tiles[(dt, tb)] = t

            # ---- v projection: v[t] = x @ wv  ([tok, vcol], xT stationary) ----
            for t in range(16):
                tb, j = t // 4, t % 4
                acc = psum.tile([128, CW], F32, name="acc_v", tag="acc")
                for dt in range(DT):
                    wvp = work.tile([128, CW], BF, name=f"wvp_{t}_{dt}",
                                    tag="p512", bufs=17)
                    nc.sync.dma_start(wvp[:], wv[dt])
                    nc.tensor.matmul(
                        acc[:],
                        xt_tiles[(dt, tb)][:, j * 128:(j + 1) * 128],
                        wvp[:],
                        start=(dt == 0), stop=(dt == DT - 1),
                    )
                nc.vector.tensor_tensor(vt[t][:], acc[:], vbias_bc[:],
                                        mybir.AluOpType.add)

            # ---- q/k projections: qT/kT[ct][col, tok] (w stationary) ----
            for ct in range(8):
                wts = []
                for dt in range(DT):
                    wt = work.tile([128, 128], BF, name=f"w_{ct}_{dt}",
                                   tag="w", bufs=32)
                    nc.sync.dma_start(wt[:], wqk[dt, ct])
                    wts.append(wt)
                dest = qT[ct] if ct < 4 else kT[ct - 4]
                for tb in range(TB):
                    acc = psum.tile([128, 512], F32, name="acc_qk", tag="acc")
                    for dt in range(DT):
                        nc.tensor.matmul(
                            acc[:], wts[dt][:], xt_tiles[(dt, tb)][:],
                            start=(dt == 0), stop=(dt == DT - 1),
                        )
                    nc.scalar.activation(
                        dest[:, tb * 512:(tb + 1) * 512], acc[:],
                        mybir.ActivationFunctionType.Identity,
                        bias=bqk_sb[ct][:], scale=1.0,
                    )

            # ---- attention + chunked AllGather ----
            for qb in range(QB):
                nk = 4 * qb + 4
                for h in range(HLOC):
                    y_ps = psum.tile([128, 512], F32, name="y_ps", tag="y")
                    sum_ps = psum.tile([1, 512], F32, name="sum_ps", tag="sums",
                                       bufs=1)
                    for kt in range(nk):
                        sc = psum.tile([128, 512], F32, name="sc", tag="s",
                                       bufs=3)
                        nc.tensor.matmul(
                            sc[:],
                            kT[h][:, kt * 128:(kt + 1) * 128],
                            qT[h][:, qb * 512:(qb + 1) * 512],
                            start=True, stop=True,
                        )
                        e = work.tile([128, 512], BF, name="expT", tag="expT",
                                      bufs=4)
                        nc.scalar.activation(
                            e[:], sc[:], mybir.ActivationFunctionType.Exp,
                            scale=SCALE,
                        )
                        p = kt - (nk - 4)
                        if p >= 0:
                            nc.vector.tensor_tensor(e[:], e[:], masks[p][:],
                                                    mybir.AluOpType.mult)
                        nc.tensor.matmul(
                            y_ps[:], vt[kt][:, h * 128:(h + 1) * 128], e[:],
                            start=(kt == 0), stop=(kt == nk - 1),
                        )
                        nc.tensor.matmul(
                            sum_ps[:], ones[:], e[:],
                            start=(kt == 0), stop=(kt == nk - 1),
                        )
                    recip = work.tile([1, 512], F32, name="recip", tag="recip",
                                      bufs=2)
                    nc.vector.reciprocal(recip[:], sum_ps[:])
                    rbc = work.tile([128, 512], F32, name="rbc", tag="rbc",
                                    bufs=2)
                    nc.gpsimd.partition_broadcast(rbc[:], recip[:], channels=128)
                    nc.vector.tensor_tensor(
                        yT[h][:, qb * 512:(qb + 1) * 512], y_ps[:], rbc[:],
                        mybir.AluOpType.mult,
                    )
                    nc.sync.dma_start(
                        ag_in[qb][h * 128:(h + 1) * 128, :],
                        yT[h][:, qb * 512:(qb + 1) * 512],
                    )
                nc.gpsimd.collective_compute(
                    "AllGather", mybir.AluOpType.bypass,
                    replica_groups=GROUPS,
                    ins=[ag_in[qb].ap()], outs=[ag_out[qb].ap()],
                )

            # ---- out projection (per AG chunk) ----
            wout_sb = []
            for dt in range(DT):
                t = work.tile([128, CW], BF, name=f"wout{dt}", tag="p512",
                              bufs=17)
                nc.sync.dma_start(t[:], wout[dt * 128:(dt + 1) * 128, :])
                wout_sb.append(t)

            for qb in range(QB):
                ygt = []
                for dt in range(DT):
                    t = work.tile([128, 512], BF, name=f"ygT_{qb}_{dt}",
                                  tag="xT", bufs=80)
                    nc.sync.dma_start(
                        t[:], ag_out[qb][dt * 128:(dt + 1) * 128, :])
                    ygt.append(t)
                for j in range(4):
                    acc = psum.tile([128, CW], F32, name="acc_o", tag="acc")
                    for dt in range(DT):
                        nc.tensor.matmul(
                            acc[:],
                            ygt[dt][:, j * 128:(j + 1) * 128],
                            wout_sb[dt][:],
                            start=(dt == 0), stop=(dt == DT - 1),
                        )
                    osb = work.tile([128, CW], F32, name="osb", tag="osb",
                                    bufs=3)
                    nc.vector.tensor_tensor(osb[:], acc[:], bias_bc[:],
                                            mybir.AluOpType.add)
                    tt = qb * 4 + j
                    nc.sync.dma_start(out[tt * 128:(tt + 1) * 128, :], osb[:])

    nc.compile()
    return nc


def _prep_inputs(x, w_qkv, b_qkv, w_out, b_out):
    """Host-side sharding/layout. Returns in_maps for the 8 cores."""
    bf16 = ml_dtypes.bfloat16
    x = np.asarray(x, dtype=np.float32)
    w_qkv = np.asarray(w_qkv, dtype=np.float32)
    b_qkv = np.asarray(b_qkv, dtype=np.float32)
    w_out = np.asarray(w_out, dtype=np.float32)
    b_out = np.asarray(b_out, dtype=np.float32)

    xT_b = [np.ascontiguousarray(x[b].T).astype(bf16) for b in range(B)]

    in_maps = []
    for c in range(8):
        b, g = c // 4, c % 4
        cols = slice(CW * g, CW * (g + 1))
        wq = w_qkv[:, 0 * D:1 * D][:, cols]
        wk = w_qkv[:, 1 * D:2 * D][:, cols]
        wv_ = w_qkv[:, 2 * D:3 * D][:, cols]
        # (dt, ct, d, col) for q(0-3) then k(4-7), 128-col tiles
        wqk = np.concatenate([wq, wk], axis=1)            # [D, 1024]
        wqk = wqk.reshape(DT, 128, 8, 128).transpose(0, 2, 1, 3)
        wqk = np.ascontiguousarray(wqk).astype(bf16)
        wv_t = np.ascontiguousarray(wv_.reshape(DT, 128, CW)).astype(bf16)

        bq = b_qkv[0 * D:1 * D][cols]
        bk = b_qkv[1 * D:2 * D][cols]
        bv_ = b_qkv[2 * D:3 * D][cols]
        bqk = np.concatenate([bq, bk]).reshape(8, 128, 1).astype(np.float32)

        in_maps.append({
            "xT": xT_b[b],
            "wqk": wqk,
            "wv": wv_t,
            "bqk": np.ascontiguousarray(bqk),
            "bv": np.ascontiguousarray(bv_.reshape(1, CW)),
            "wout": np.ascontiguousarray(w_out[:, cols]).astype(bf16),
            "bout": np.ascontiguousarray(b_out[cols].reshape(1, CW)),
        })
    return in_maps


def kernel(x, w_qkv, b_qkv, w_out, b_out, _trace=False, _trace_kwargs=None):
    from concourse.bass_utils import run_bass_kernel_spmd

    if "nc" not in _cache:
        _cache["nc"] = _build()
    nc = _cache["nc"]

    in_maps = _prep_inputs(x, w_qkv, b_qkv, w_out, b_out)
    res = run_bass_kernel_spmd(
        nc, in_maps, core_ids=list(range(8)),
        trace=_trace, **(_trace_kwargs or {}),
    )

    out = np.empty((B, S, D), dtype=np.float32)
    for c in range(8):
        b, g = c // 4, c % 4
        out[b][:, CW * g:CW * (g + 1)] = res.results[c]["out"]
    kernel.last_result = res
    return out


# revision 10
# speedup vs baseline: 1.1725x; 1.1725x over previous
"""Causal self-attention (B=2, S=2048, D=2048, H=16) on 8 TRN2 NeuronCores.

Sharding: 2 batches x 4 head-groups.  Core c handles batch c//4 and heads
[4*(c%4) .. 4*(c%4)+3].  Per core:
  phase 1: qT/kT (transposed) + v projections from host-pre-transposed xT
  phase 2: causal attention per (q-block, head), q-blocks processed in
           descending order so each q-block's AllGather (groups of 4) of the
           normalized yT overlaps the remaining attention + out-projection
  phase 3: out projection of the core's 512 output columns + bias
Softmax uses exp without max subtraction (logits are O(8) here); denominators
are accumulated on DVE (elementwise over k-tiles) then reduced across
partitions with a single ones-matmul, inverted with reciprocal_approx_fast.

Compute is bf16 with fp32 PSUM accumulation; measured l2 rel err vs the fp32
reference is ~5.5e-3.
"""

import numpy as np
import ml_dtypes

B, S, D = 2, 2048, 2048
H, HD = 16, 128
HLOC = 4           # heads per core
CW = HLOC * HD     # 512: per-core q/k/v width and out-column width
QB = 4             # q blocks of 512
DT = 16            # d tiles of 128
TB = 4             # token blocks of 512
SCALE = 1.0 / float(np.sqrt(HD))
GROUPS = [[0, 1, 2, 3], [4, 5, 6, 7]]

_cache = {}


def _build():
    import concourse.tile as tile
    import concourse.mybir as mybir
    from concourse import bacc

    BF = mybir.dt.bfloat16
    F32 = mybir.dt.float32

    nc = bacc.Bacc("TRN2", target_bir_lowering=False, debug=False, num_devices=8)

    # Inputs (per-core shards, host-prepared)
    xT = nc.dram_tensor("xT", [D, S], BF, kind="ExternalInput")            # x[batch].T
    wqk = nc.dram_tensor("wqk", [DT, 8, 128, 128], BF, kind="ExternalInput")  # (dt, ct, d, col)
    wv = nc.dram_tensor("wv", [DT, 128, CW], BF, kind="ExternalInput")     # (dt, d, vcol)
    bqk = nc.dram_tensor("bqk", [8, 128, 1], F32, kind="ExternalInput")
    bv = nc.dram_tensor("bv", [1, CW], F32, kind="ExternalInput")
    wout = nc.dram_tensor("wout", [D, CW], BF, kind="ExternalInput")
    bout = nc.dram_tensor("bout", [1, CW], F32, kind="ExternalInput")
    out = nc.dram_tensor("out", [S, CW], F32, kind="ExternalOutput")

    ag_in = [nc.dram_tensor(f"ag_in{qb}", [CW, 512], BF, kind="Internal")
             for qb in range(QB)]
    ag_out = [nc.dram_tensor(f"ag_out{qb}", [D, 512], BF, kind="Internal")
              for qb in range(QB)]

    with tile.TileContext(nc) as tc:
        with (
            tc.tile_pool(name="const", bufs=1) as constp,
            tc.tile_pool(name="pers", bufs=1) as pers,
            tc.tile_pool(name="work", bufs=2) as work,
            tc.tile_pool(name="psum", bufs=2, space="PSUM") as psum,
        ):
            # ---- constants ----
            ones = constp.tile([128, 1], BF, name="ones")
            nc.gpsimd.memset(ones[:], 1.0)

            masks = []
            for p in range(4):
                m = constp.tile([128, 512], BF, name=f"mask{p}", tag=f"mask{p}")
                nc.gpsimd.memset(m[:], 1.0)
                # keep (1.0) where qq >= kk + 128*p, else 0.0
                nc.gpsimd.affine_select(
                    out=m[:], in_=m[:],
                    compare_op=mybir.AluOpType.is_ge, fill=0.0,
                    base=-128 * p, channel_multiplier=-1, pattern=[[1, 512]],
                )
                masks.append(m)

            bout_sb = constp.tile([1, CW], F32, name="bout_sb")
            nc.sync.dma_start(bout_sb[:], bout[:])
            bias_bc = constp.tile([128, CW], F32, name="bias_bc")
            nc.gpsimd.partition_broadcast(bias_bc[:], bout_sb[:], channels=128)

            bv_sb = constp.tile([1, CW], F32, name="bv_sb")
            nc.sync.dma_start(bv_sb[:], bv[:])
            vbias_bc = constp.tile([128, CW], F32, name="vbias_bc")
            nc.gpsimd.partition_broadcast(vbias_bc[:], bv_sb[:], channels=128)

            bqk_sb = []
            for ct in range(8):
                t = constp.tile([128, 1], F32, name=f"bqk{ct}", tag=f"bqk{ct}")
                nc.sync.dma_start(t[:], bqk[ct])
                bqk_sb.append(t)

            # ---- persistent SBUF tensors ----
            qT = [pers.tile([128, S], BF, name=f"qT{h}", tag=f"qT{h}")
                  for h in range(HLOC)]
            kT = [pers.tile([128, S], BF, name=f"kT{h}", tag=f"kT{h}")
                  for h in range(HLOC)]
            vt = [pers.tile([128, CW], BF, name=f"v{t}", tag=f"v{t}")
                  for t in range(16)]
            yT = [pers.tile([128, S], BF, name=f"yT{h}", tag=f"yT{h}")
                  for h in range(HLOC)]

            # ---- load xT (64 tiles [128 d, 512 tok]); tb-major so compute
            # can start after the first token block arrives ----
            xt_tiles = {}
            for tb in range(TB):
                for dt in range(DT):
                    t = work.tile([128, 512], BF, name=f"xt_{dt}_{tb}",
                                  tag="xT", bufs=68)
                    nc.sync.dma_start(
                        t[:], xT[dt * 128:(dt + 1) * 128, tb * 512:(tb + 1) * 512])
                    xt_tiles[(dt, tb)] = t

            # ---- v projection: v[t] = x @ wv  ([tok, vcol], xT stationary) ----
            wv_sb = []
            for dt in range(DT):
                wvp = work.tile([128, CW], BF, name=f"wvp{dt}", tag="p512",
                                bufs=17)
                nc.sync.dma_start(wvp[:], wv[dt])
                wv_sb.append(wvp)
            for t in range(16):
                tb, j = t // 4, t % 4
                acc = psum.tile([128, CW], F32, name="acc_v", tag="acc", bufs=3)
                for dt in range(DT):
                    nc.tensor.matmul(
                        acc[:],
                        xt_tiles[(dt, tb)][:, j * 128:(j + 1) * 128],
                        wv_sb[dt][:],
                        start=(dt == 0), stop=(dt == DT - 1),
                    )
                nc.vector.tensor_tensor(vt[t][:], acc[:], vbias_bc[:],
                                        mybir.AluOpType.add)

            # ---- q/k projections: qT/kT[ct][col, tok] (w stationary) ----
            for ct in range(8):
                wts = []
                for dt in range(DT):
                    wt = work.tile([128, 128], BF, name=f"w_{ct}_{dt}",
                                   tag="w", bufs=24)
                    nc.sync.dma_start(wt[:], wqk[dt, ct])
                    wts.append(wt)
                dest = qT[ct] if ct < 4 else kT[ct - 4]
                for tb in range(TB):
                    acc = psum.tile([128, 512], F32, name="acc_qk", tag="acc",
                                    bufs=3)
                    for dt in range(DT):
                        nc.tensor.matmul(
                            acc[:], wts[dt][:], xt_tiles[(dt, tb)][:],
                            start=(dt == 0), stop=(dt == DT - 1),
                        )
                    nc.scalar.activation(
                        dest[:, tb * 512:(tb + 1) * 512], acc[:],
                        mybir.ActivationFunctionType.Identity,
                        bias=bqk_sb[ct][:], scale=1.0,
                    )

            # ---- attention + chunked AllGather (qb descending) ----
            for qb in (3, 2, 1, 0):
                nk = 4 * qb + 4
                for h in range(HLOC):
                    y_ps = psum.tile([128, 512], F32, name="y_ps", tag="y")
                    esum = work.tile([128, 512], F32, name="esum", tag="esum",
                                     bufs=2)
                    for kt in range(nk):
                        sc = psum.tile([128, 512], F32, name="sc", tag="s",
                                       bufs=3)
                        nc.tensor.matmul(
                            sc[:],
                            kT[h][:, kt * 128:(kt + 1) * 128],
                            qT[h][:, qb * 512:(qb + 1) * 512],
                            start=True, stop=True,
                        )
                        e = work.tile([128, 512], BF, name="expT", tag="expT",
                                      bufs=5)
                        nc.scalar.activation(
                            e[:], sc[:], mybir.ActivationFunctionType.Exp,
                            scale=SCALE,
                        )
                        p = kt - (nk - 4)
                        if p >= 0:
                            nc.vector.tensor_tensor(e[:], e[:], masks[p][:],
                                                    mybir.AluOpType.mult)
                        nc.tensor.matmul(
                            y_ps[:], vt[kt][:, h * 128:(h + 1) * 128], e[:],
                            start=(kt == 0), stop=(kt == nk - 1),
                        )
                        if kt == 0:
                            nc.vector.tensor_copy(esum[:], e[:])
                        else:
                            nc.vector.tensor_tensor(esum[:], esum[:], e[:],
                                                    mybir.AluOpType.add)
                    esum_bf = work.tile([128, 512], BF, name="esum_bf",
                                        tag="esum_bf", bufs=2)
                    nc.vector.tensor_copy(esum_bf[:], esum[:])
                    sum_ps = psum.tile([1, 512], F32, name="sum_ps", tag="s",
                                       bufs=3)
                    nc.tensor.matmul(sum_ps[:], ones[:], esum_bf[:],
                                     start=True, stop=True)
                    recip = work.tile([1, 512], F32, name="recip", tag="recip",
                                      bufs=2)
                    nc.vector.reciprocal_approx_fast(recip[:], sum_ps[:])
                    rbc = work.tile([128, 512], F32, name="rbc", tag="rbc",
                                    bufs=2)
                    nc.gpsimd.partition_broadcast(rbc[:], recip[:], channels=128)
                    nc.vector.tensor_tensor(
                        yT[h][:, qb * 512:(qb + 1) * 512], y_ps[:], rbc[:],
                        mybir.AluOpType.mult,
                    )
                    nc.sync.dma_start(
                        ag_in[qb][h * 128:(h + 1) * 128, :],
                        yT[h][:, qb * 512:(qb + 1) * 512],
                    )
                nc.gpsimd.collective_compute(
                    "AllGather", mybir.AluOpType.bypass,
                    replica_groups=GROUPS,
                    ins=[ag_in[qb].ap()], outs=[ag_out[qb].ap()],
                )

            # ---- out projection (per AG chunk, matching AG order) ----
            wout_sb = []
            for dt in range(DT):
                t = work.tile([128, CW], BF, name=f"wout{dt}", tag="p512",
                              bufs=17)
                nc.sync.dma_start(t[:], wout[dt * 128:(dt + 1) * 128, :])
                wout_sb.append(t)

            for qb in (3, 2, 1, 0):
                ygt = []
                for dt in range(DT):
                    t = work.tile([128, 512], BF, name=f"ygT_{qb}_{dt}",
                                  tag="xT", bufs=68)
                    nc.sync.dma_start(
                        t[:], ag_out[qb][dt * 128:(dt + 1) * 128, :])
                    ygt.append(t)
                for j in range(4):
                    acc = psum.tile([128, CW], F32, name="acc_o", tag="acc",
                                    bufs=3)
                    for dt in range(DT):
                        nc.tensor.matmul(
                            acc[:],
                            ygt[dt][:, j * 128:(j + 1) * 128],
                            wout_sb[dt][:],
                            start=(dt == 0), stop=(dt == DT - 1),
                        )
                    osb = work.tile([128, CW], F32, name="osb", tag="osb",
                                    bufs=3)
                    nc.vector.tensor_tensor(osb[:], acc[:], bias_bc[:],
                                            mybir.AluOpType.add)
                    tt = qb * 4 + j
                    nc.sync.dma_start(out[tt * 128:(tt + 1) * 128, :], osb[:])

    nc.compile()
    return nc


def _prep_inputs(x, w_qkv, b_qkv, w_out, b_out):
    """Host-side sharding/layout. Returns in_maps for the 8 cores."""
    bf16 = ml_dtypes.bfloat16
    x = np.asarray(x, dtype=np.float32)
    w_qkv = np.asarray(w_qkv, dtype=np.float32)
    b_qkv = np.asarray(b_qkv, dtype=np.float32)
    w_out = np.asarray(w_out, dtype=np.float32)
    b_out = np.asarray(b_out, dtype=np.float32)

    xT_b = [np.ascontiguousarray(x[b].T).astype(bf16) for b in range(B)]

    in_maps = []
    for c in range(8):
        b, g = c // 4, c % 4
        cols = slice(CW * g, CW * (g + 1))
        wq = w_qkv[:, 0 * D:1 * D][:, cols]
        wk = w_qkv[:, 1 * D:2 * D][:, cols]
        wv_ = w_qkv[:, 2 * D:3 * D][:, cols]
        # (dt, ct, d, col) for q(0-3) then k(4-7), 128-col tiles
        wqk = np.concatenate([wq, wk], axis=1)            # [D, 1024]
        wqk = wqk.reshape(DT, 128, 8, 128).transpose(0, 2, 1, 3)
        wqk = np.ascontiguousarray(wqk).astype(bf16)
        wv_t = np.ascontiguousarray(wv_.reshape(DT, 128, CW)).astype(bf16)

        bq = b_qkv[0 * D:1 * D][cols]
        bk = b_qkv[1 * D:2 * D][cols]
        bv_ = b_qkv[2 * D:3 * D][cols]
        bqk = np.concatenate([bq, bk]).reshape(8, 128, 1).astype(np.float32)

        in_maps.append({
            "xT": xT_b[b],
            "wqk": wqk,
            "wv": wv_t,
            "bqk": np.ascontiguousarray(bqk),
            "bv": np.ascontiguousarray(bv_.reshape(1, CW)),
            "wout": np.ascontiguousarray(w_out[:, cols]).astype(bf16),
            "bout": np.ascontiguousarray(b_out[cols].reshape(1, CW)),
        })
    return in_maps


def kernel(x, w_qkv, b_qkv, w_out, b_out, _trace=False, _trace_kwargs=None):
    from concourse.bass_utils import run_bass_kernel_spmd

    if "nc" not in _cache:
        _cache["nc"] = _build()
    nc = _cache["nc"]

    in_maps = _prep_inputs(x, w_qkv, b_qkv, w_out, b_out)
    res = run_bass_kernel_spmd(
        nc, in_maps, core_ids=list(range(8)),
        trace=_trace, **(_trace_kwargs or {}),
    )

    out = np.empty((B, S, D), dtype=np.float32)
    for c in range(8):
        b, g = c // 4, c % 4
        out[b][:, CW * g:CW * (g + 1)] = res.results[c]["out"]
    kernel.last_result = res
    return out


# revision 16
# speedup vs baseline: 1.2590x; 1.0738x over previous
"""Causal self-attention (B=2, S=2048, D=2048, H=16) on 8 TRN2 NeuronCores.

Sharding: 2 batches x 4 head-groups.  Core c handles batch c//4 and heads
[4*(c%4) .. 4*(c%4)+3].  Per core:
  phase 1: qT/kT (transposed) + v projections from host-pre-transposed xT
  phase 2: causal attention per (q-block, head), q-blocks processed in
           descending order so each q-block's AllGather (groups of 4) of the
           normalized yT overlaps the remaining attention + out-projection
  phase 3: out projection of the core's 512 output columns + bias
Softmax uses exp without max subtraction (logits are O(8) here); denominators
are accumulated on DVE (elementwise over k-tiles) then reduced across
partitions with a single ones-matmul, inverted with reciprocal_approx_fast.

Compute is bf16 with fp32 PSUM accumulation; measured l2 rel err vs the fp32
reference is ~5.5e-3.
"""

import numpy as np
import ml_dtypes

B, S, D = 2, 2048, 2048
H, HD = 16, 128
HLOC = 4           # heads per core
CW = HLOC * HD     # 512: per-core q/k/v width and out-column width
QB = 4             # q blocks of 512
DT = 16            # d tiles of 128
TB = 4             # token blocks of 512
SCALE = 1.0 / float(np.sqrt(HD))
GROUPS = [[0, 1, 2, 3], [4, 5, 6, 7]]

_cache = {}


def _build():
    import concourse.tile as tile
    import concourse.mybir as mybir
    from concourse import bacc

    BF = mybir.dt.bfloat16
    F32 = mybir.dt.float32

    nc = bacc.Bacc("TRN2", target_bir_lowering=False, debug=False, num_devices=8)

    # Inputs (per-core shards, host-prepared)
    xT = nc.dram_tensor("xT", [D, S], BF, kind="ExternalInput")            # x[batch].T
    wqk = nc.dram_tensor("wqk", [DT, 8, 128, 128], BF, kind="ExternalInput")  # (dt, ct, d, col)
    wv = nc.dram_tensor("wv", [DT, 128, CW], BF, kind="ExternalInput")     # (dt, d, vcol)
    bqk = nc.dram_tensor("bqk", [8, 128, 1], F32, kind="ExternalInput")
    bv = nc.dram_tensor("bv", [1, CW], F32, kind="ExternalInput")
    wout = nc.dram_tensor("wout", [D, CW], BF, kind="ExternalInput")
    bout = nc.dram_tensor("bout", [1, CW], F32, kind="ExternalInput")
    out = nc.dram_tensor("out", [S, CW], F32, kind="ExternalOutput")

    ag_in = [nc.dram_tensor(f"ag_in{qb}", [CW, 512], BF, kind="Internal")
             for qb in range(QB)]
    ag_out = [nc.dram_tensor(f"ag_out{qb}", [D, 512], BF, kind="Internal")
              for qb in range(QB)]

    with tile.TileContext(nc) as tc:
        with (
            tc.tile_pool(name="const", bufs=1) as constp,
            tc.tile_pool(name="pers", bufs=1) as pers,
            tc.tile_pool(name="work", bufs=2) as work,
            tc.tile_pool(name="psum", bufs=2, space="PSUM") as psum,
        ):
            # ---- constants ----
            ones = constp.tile([128, 1], BF, name="ones")
            nc.gpsimd.memset(ones[:], 1.0)

            # Pair masks for the 4 diagonal k-subtiles, packed two subtiles
            # wide: pairmask[m][:, 512*sub + qq] keeps where
            # qq >= kk + 128*(2m+sub).
            pairmasks = []
            for pm in range(2):
                m = constp.tile([128, 1024], BF, name=f"pmask{pm}",
                                tag=f"pmask{pm}")
                nc.gpsimd.memset(m[:], 1.0)
                for sub in range(2):
                    nc.gpsimd.affine_select(
                        out=m[:, sub * 512:(sub + 1) * 512],
                        in_=m[:, sub * 512:(sub + 1) * 512],
                        compare_op=mybir.AluOpType.is_ge, fill=0.0,
                        base=-128 * (2 * pm + sub), channel_multiplier=-1,
                        pattern=[[1, 512]],
                    )
                pairmasks.append(m)

            bout_sb = constp.tile([1, CW], F32, name="bout_sb")
            nc.sync.dma_start(bout_sb[:], bout[:])
            bias_bc = constp.tile([128, CW], F32, name="bias_bc")
            nc.gpsimd.partition_broadcast(bias_bc[:], bout_sb[:], channels=128)

            bv_sb = constp.tile([1, CW], F32, name="bv_sb")
            nc.sync.dma_start(bv_sb[:], bv[:])
            vbias_bc = constp.tile([128, CW], F32, name="vbias_bc")
            nc.gpsimd.partition_broadcast(vbias_bc[:], bv_sb[:], channels=128)

            bqk_sb = []
            for ct in range(8):
                t = constp.tile([128, 1], F32, name=f"bqk{ct}", tag=f"bqk{ct}")
                nc.sync.dma_start(t[:], bqk[ct])
                bqk_sb.append(t)

            # ---- persistent SBUF tensors ----
            qT = [pers.tile([128, S], BF, name=f"qT{h}", tag=f"qT{h}")
                  for h in range(HLOC)]
            kT = [pers.tile([128, S], BF, name=f"kT{h}", tag=f"kT{h}")
                  for h in range(HLOC)]
            vt = [pers.tile([128, CW], BF, name=f"v{t}", tag=f"v{t}")
                  for t in range(16)]
            yT = [pers.tile([128, S], BF, name=f"yT{h}", tag=f"yT{h}")
                  for h in range(HLOC)]

            # ---- load xT (64 tiles [128 d, 512 tok]); tb-major so compute
            # can start after the first token block arrives ----
            xt_tiles = {}
            for tb in range(TB):
                for dt in range(DT):
                    t = work.tile([128, 512], BF, name=f"xt_{dt}_{tb}",
                                  tag="xT", bufs=68)
                    nc.sync.dma_start(
                        t[:], xT[dt * 128:(dt + 1) * 128, tb * 512:(tb + 1) * 512])
                    xt_tiles[(dt, tb)] = t

            # ---- v projection: v[t] = x @ wv  ([tok, vcol], xT stationary) ----
            wv_sb = []
            for dt in range(DT):
                wvp = work.tile([128, CW], BF, name=f"wvp{dt}", tag="p512",
                                bufs=17)
                nc.sync.dma_start(wvp[:], wv[dt])
                wv_sb.append(wvp)
            for t in range(16):
                tb, j = t // 4, t % 4
                acc = psum.tile([128, CW], F32, name="acc_v", tag="acc", bufs=2)
                for dt in range(DT):
                    nc.tensor.matmul(
                        acc[:],
                        xt_tiles[(dt, tb)][:, j * 128:(j + 1) * 128],
                        wv_sb[dt][:],
                        start=(dt == 0), stop=(dt == DT - 1),
                    )
                nc.vector.tensor_tensor(vt[t][:], acc[:], vbias_bc[:],
                                        mybir.AluOpType.add)

            # ---- q/k projections: qT/kT[ct][col, tok] (w stationary) ----
            for ct in range(8):
                wts = []
                for dt in range(DT):
                    wt = work.tile([128, 128], BF, name=f"w_{ct}_{dt}",
                                   tag="w", bufs=24)
                    nc.sync.dma_start(wt[:], wqk[dt, ct])
                    wts.append(wt)
                dest = qT[ct] if ct < 4 else kT[ct - 4]
                for tb in range(TB):
                    acc = psum.tile([128, 512], F32, name="acc_qk", tag="acc",
                                    bufs=2)
                    for dt in range(DT):
                        nc.tensor.matmul(
                            acc[:], wts[dt][:], xt_tiles[(dt, tb)][:],
                            start=(dt == 0), stop=(dt == DT - 1),
                        )
                    nc.scalar.activation(
                        dest[:, tb * 512:(tb + 1) * 512], acc[:],
                        mybir.ActivationFunctionType.Identity,
                        bias=bqk_sb[ct][:], scale=1.0,
                    )

            # ---- attention + chunked AllGather (qb descending), with the
            # out-projection of chunk qb+1 interleaved one AG behind so the
            # sync DMA queue never head-of-line blocks on a collective ----
            wout_sb = []
            for dt in range(DT):
                t = work.tile([128, CW], BF, name=f"wout{dt}", tag="p512",
                              bufs=17)
                nc.sync.dma_start(t[:], wout[dt * 128:(dt + 1) * 128, :])
                wout_sb.append(t)

            def attention_chunk(qb):
                nk = 4 * qb + 4
                for h in range(HLOC):
                    y_ps = psum.tile([128, 512], F32, name="y_ps", tag="y")
                    esum = work.tile([128, 1024], BF, name="esum", tag="esum",
                                     bufs=2)
                    # k-subtiles in pairs; PV/sum of pair pr-1 issue after
                    # QK/exp of pair pr so the PE never waits on a fresh exp
                    prev = None

                    def flush(prev_pair):
                        e, pr = prev_pair
                        for s_ in range(2):
                            kt = 2 * pr + s_
                            nc.tensor.matmul(
                                y_ps[:],
                                vt[kt][:, h * 128:(h + 1) * 128],
                                e[:, s_ * 512:(s_ + 1) * 512],
                                start=(kt == 0), stop=(kt == nk - 1),
                            )
                        if pr == 0:
                            nc.vector.tensor_copy(esum[:], e[:])
                        else:
                            nc.vector.tensor_tensor(esum[:], esum[:], e[:],
                                                    mybir.AluOpType.add)

                    for pr in range(nk // 2):
                        sc = psum.tile([128, 1024], F32, name="sc", tag="s",
                                       bufs=2)
                        for s_ in range(2):
                            kt = 2 * pr + s_
                            nc.tensor.matmul(
                                sc[:, s_ * 512:(s_ + 1) * 512],
                                kT[h][:, kt * 128:(kt + 1) * 128],
                                qT[h][:, qb * 512:(qb + 1) * 512],
                                start=True, stop=True,
                            )
                        e = work.tile([128, 1024], BF, name="expT", tag="expT",
                                      bufs=4)
                        nc.scalar.activation(
                            e[:], sc[:], mybir.ActivationFunctionType.Exp,
                            scale=SCALE,
                        )
                        pm = pr - (nk // 2 - 2)
                        if pm >= 0:
                            nc.vector.tensor_tensor(e[:], e[:],
                                                    pairmasks[pm][:],
                                                    mybir.AluOpType.mult)
                        if prev is not None:
                            flush(prev)
                        prev = (e, pr)
                    flush(prev)

                    esum_f = work.tile([128, 512], BF, name="esum_f",
                                       tag="esum_f", bufs=2)
                    nc.vector.tensor_tensor(esum_f[:], esum[:, 0:512],
                                            esum[:, 512:1024],
                                            mybir.AluOpType.add)
                    sum_ps = psum.tile([1, 512], F32, name="sum_ps", tag="y")
                    nc.tensor.matmul(sum_ps[:], ones[:], esum_f[:],
                                     start=True, stop=True)
                    recip = work.tile([1, 512], F32, name="recip", tag="recip",
                                      bufs=2)
                    nc.vector.reciprocal_approx_fast(recip[:], sum_ps[:])
                    rbc = work.tile([128, 512], F32, name="rbc", tag="rbc",
                                    bufs=2)
                    nc.gpsimd.partition_broadcast(rbc[:], recip[:], channels=128)
                    nc.vector.tensor_tensor(
                        yT[h][:, qb * 512:(qb + 1) * 512], y_ps[:], rbc[:],
                        mybir.AluOpType.mult,
                    )
                    nc.sync.dma_start(
                        ag_in[qb][h * 128:(h + 1) * 128, :],
                        yT[h][:, qb * 512:(qb + 1) * 512],
                    )
                nc.gpsimd.collective_compute(
                    "AllGather", mybir.AluOpType.bypass,
                    replica_groups=GROUPS,
                    ins=[ag_in[qb].ap()], outs=[ag_out[qb].ap()],
                )

            def outproj_chunk(qb):
                ygt = []
                for dt in range(DT):
                    t = work.tile([128, 512], BF, name=f"ygT_{qb}_{dt}",
                                  tag="xT", bufs=68)
                    nc.sync.dma_start(
                        t[:], ag_out[qb][dt * 128:(dt + 1) * 128, :])
                    ygt.append(t)
                for j in range(4):
                    acc = psum.tile([128, CW], F32, name="acc_o", tag="acc",
                                    bufs=2)
                    for dt in range(DT):
                        nc.tensor.matmul(
                            acc[:],
                            ygt[dt][:, j * 128:(j + 1) * 128],
                            wout_sb[dt][:],
                            start=(dt == 0), stop=(dt == DT - 1),
                        )
                    osb = work.tile([128, CW], F32, name="osb", tag="osb",
                                    bufs=3)
                    nc.vector.tensor_tensor(osb[:], acc[:], bias_bc[:],
                                            mybir.AluOpType.add)
                    tt = qb * 4 + j
                    nc.sync.dma_start(out[tt * 128:(tt + 1) * 128, :], osb[:])

            attention_chunk(3)
            attention_chunk(2)
            outproj_chunk(3)
            attention_chunk(1)
            outproj_chunk(2)
            attention_chunk(0)
            outproj_chunk(1)
            outproj_chunk(0)

    nc.compile()
    return nc


def _prep_inputs(x, w_qkv, b_qkv, w_out, b_out):
    """Host-side sharding/layout. Returns in_maps for the 8 cores."""
    bf16 = ml_dtypes.bfloat16
    x = np.asarray(x, dtype=np.float32)
    w_qkv = np.asarray(w_qkv, dtype=np.float32)
    b_qkv = np.asarray(b_qkv, dtype=np.float32)
    w_out = np.asarray(w_out, dtype=np.float32)
    b_out = np.asarray(b_out, dtype=np.float32)

    xT_b = [np.ascontiguousarray(x[b].T).astype(bf16) for b in range(B)]

    in_maps = []
    for c in range(8):
        b, g = c // 4, c % 4
        cols = slice(CW * g, CW * (g + 1))
        wq = w_qkv[:, 0 * D:1 * D][:, cols]
        wk = w_qkv[:, 1 * D:2 * D][:, cols]
        wv_ = w_qkv[:, 2 * D:3 * D][:, cols]
        # (dt, ct, d, col) for q(0-3) then k(4-7), 128-col tiles
        wqk = np.concatenate([wq, wk], axis=1)            # [D, 1024]
        wqk = wqk.reshape(DT, 128, 8, 128).transpose(0, 2, 1, 3)
        wqk = np.ascontiguousarray(wqk).astype(bf16)
        wv_t = np.ascontiguousarray(wv_.reshape(DT, 128, CW)).astype(bf16)

        bq = b_qkv[0 * D:1 * D][cols]
        bk = b_qkv[1 * D:2 * D][cols]
        bv_ = b_qkv[2 * D:3 * D][cols]
        bqk = np.concatenate([bq, bk]).reshape(8, 128, 1).astype(np.float32)

        in_maps.append({
            "xT": xT_b[b],
            "wqk": wqk,
            "wv": wv_t,
            "bqk": np.ascontiguousarray(bqk),
            "bv": np.ascontiguousarray(bv_.reshape(1, CW)),
            "wout": np.ascontiguousarray(w_out[:, cols]).astype(bf16),
            "bout": np.ascontiguousarray(b_out[cols].reshape(1, CW)),
        })
    return in_maps


def kernel(x, w_qkv, b_qkv, w_out, b_out, _trace=False, _trace_kwargs=None):
    from concourse.bass_utils import run_bass_kernel_spmd

    if "nc" not in _cache:
        _cache["nc"] = _build()
    nc = _cache["nc"]

    in_maps = _prep_inputs(x, w_qkv, b_qkv, w_out, b_out)
    res = run_bass_kernel_spmd(
        nc, in_maps, core_ids=list(range(8)),
        trace=_trace, **(_trace_kwargs or {}),
    )

    out = np.empty((B, S, D), dtype=np.float32)
    for c in range(8):
        b, g = c // 4, c % 4
        out[b][:, CW * g:CW * (g + 1)] = res.results[c]["out"]
    kernel.last_result = res
    return out


# revision 17
# speedup vs baseline: 1.3503x; 1.0726x over previous
"""Causal self-attention (B=2, S=2048, D=2048, H=16) on 8 TRN2 NeuronCores.

Sharding: 2 batches x 4 head-groups.  Core c handles batch c//4 and heads
[4*(c%4) .. 4*(c%4)+3].  Per core:
  phase 1: qT/kT (transposed) + v projections from host-pre-transposed xT
  phase 2: causal attention per (q-block, head), q-blocks processed in
           descending order so each q-block's AllGather (groups of 4) of the
           normalized yT overlaps the remaining attention + out-projection
  phase 3: out projection of the core's 512 output columns + bias
Softmax uses exp without max subtraction (logits are O(8) here); denominators
are accumulated on DVE (elementwise over k-tiles) then reduced across
partitions with a single ones-matmul, inverted with reciprocal_approx_fast.

Compute is bf16 with fp32 PSUM accumulation; measured l2 rel err vs the fp32
reference is ~5.5e-3.
"""

import numpy as np
import ml_dtypes

B, S, D = 2, 2048, 2048
H, HD = 16, 128
HLOC = 4           # heads per core
CW = HLOC * HD     # 512: per-core q/k/v width and out-column width
QB = 4             # q blocks of 512
DT = 16            # d tiles of 128
TB = 4             # token blocks of 512
SCALE = 1.0 / float(np.sqrt(HD))
GROUPS = [[0, 1, 2, 3], [4, 5, 6, 7]]

_cache = {}


def _build():
    import concourse.tile as tile
    import concourse.mybir as mybir
    from concourse import bacc

    BF = mybir.dt.bfloat16
    F32 = mybir.dt.float32

    nc = bacc.Bacc("TRN2", target_bir_lowering=False, debug=False, num_devices=8)

    # Inputs (per-core shards, host-prepared)
    xT = nc.dram_tensor("xT", [D, S], BF, kind="ExternalInput")            # x[batch].T
    wqk = nc.dram_tensor("wqk", [DT, 8, 128, 128], BF, kind="ExternalInput")  # (dt, ct, d, col)
    wv = nc.dram_tensor("wv", [DT, 128, CW], BF, kind="ExternalInput")     # (dt, d, vcol)
    bqk = nc.dram_tensor("bqk", [8, 128, 1], F32, kind="ExternalInput")
    bv = nc.dram_tensor("bv", [1, CW], F32, kind="ExternalInput")
    wout = nc.dram_tensor("wout", [D, CW], BF, kind="ExternalInput")
    bout = nc.dram_tensor("bout", [1, CW], F32, kind="ExternalInput")
    out = nc.dram_tensor("out", [S, CW], F32, kind="ExternalOutput")

    ag_in = [nc.dram_tensor(f"ag_in{qb}", [CW, 512], BF, kind="Internal")
             for qb in range(QB)]
    ag_out = [nc.dram_tensor(f"ag_out{qb}", [D, 512], BF, kind="Internal")
              for qb in range(QB)]

    with tile.TileContext(nc) as tc:
        with (
            tc.tile_pool(name="const", bufs=1) as constp,
            tc.tile_pool(name="pers", bufs=1) as pers,
            tc.tile_pool(name="work", bufs=2) as work,
            tc.tile_pool(name="psum", bufs=2, space="PSUM") as psum,
        ):
            # ---- constants ----
            ones = constp.tile([128, 1], BF, name="ones")
            nc.gpsimd.memset(ones[:], 1.0)

            # Pair masks for the 4 diagonal k-subtiles, packed two subtiles
            # wide: pairmask[m][:, 512*sub + qq] keeps where
            # qq >= kk + 128*(2m+sub).
            pairmasks = []
            for pm in range(2):
                m = constp.tile([128, 1024], BF, name=f"pmask{pm}",
                                tag=f"pmask{pm}")
                nc.gpsimd.memset(m[:], 1.0)
                for sub in range(2):
                    nc.gpsimd.affine_select(
                        out=m[:, sub * 512:(sub + 1) * 512],
                        in_=m[:, sub * 512:(sub + 1) * 512],
                        compare_op=mybir.AluOpType.is_ge, fill=0.0,
                        base=-128 * (2 * pm + sub), channel_multiplier=-1,
                        pattern=[[1, 512]],
                    )
                pairmasks.append(m)

            bout_sb = constp.tile([1, CW], F32, name="bout_sb")
            nc.sync.dma_start(bout_sb[:], bout[:])
            bias_bc = constp.tile([128, CW], F32, name="bias_bc")
            nc.gpsimd.partition_broadcast(bias_bc[:], bout_sb[:], channels=128)

            bv_sb = constp.tile([1, CW], F32, name="bv_sb")
            nc.sync.dma_start(bv_sb[:], bv[:])
            vbias_bc = constp.tile([128, CW], F32, name="vbias_bc")
            nc.gpsimd.partition_broadcast(vbias_bc[:], bv_sb[:], channels=128)

            bqk_sb = []
            for ct in range(8):
                t = constp.tile([128, 1], F32, name=f"bqk{ct}", tag=f"bqk{ct}")
                nc.sync.dma_start(t[:], bqk[ct])
                bqk_sb.append(t)

            # ---- persistent SBUF tensors ----
            qT = [pers.tile([128, S], BF, name=f"qT{h}", tag=f"qT{h}")
                  for h in range(HLOC)]
            kT = [pers.tile([128, S], BF, name=f"kT{h}", tag=f"kT{h}")
                  for h in range(HLOC)]
            vt = [pers.tile([128, CW], BF, name=f"v{t}", tag=f"v{t}")
                  for t in range(16)]
            yT = [pers.tile([128, S], BF, name=f"yT{h}", tag=f"yT{h}")
                  for h in range(HLOC)]

            # ---- input loads: wv panels first (v-projection can then start
            # as soon as the first xT token block lands), then xT tb-major ----
            wv_sb = []
            for dt in range(DT):
                wvp = work.tile([128, CW], BF, name=f"wvp{dt}", tag="p512",
                                bufs=17)
                nc.sync.dma_start(wvp[:], wv[dt])
                wv_sb.append(wvp)
            xt_tiles = {}
            for tb in range(TB):
                for dt in range(DT):
                    t = work.tile([128, 512], BF, name=f"xt_{dt}_{tb}",
                                  tag="xT", bufs=68)
                    nc.sync.dma_start(
                        t[:], xT[dt * 128:(dt + 1) * 128, tb * 512:(tb + 1) * 512])
                    xt_tiles[(dt, tb)] = t

            # ---- v projection: v[t] = x @ wv  ([tok, vcol], xT stationary) ----
            for t in range(16):
                tb, j = t // 4, t % 4
                acc = psum.tile([128, CW], F32, name="acc_v", tag="acc", bufs=2)
                for dt in range(DT):
                    nc.tensor.matmul(
                        acc[:],
                        xt_tiles[(dt, tb)][:, j * 128:(j + 1) * 128],
                        wv_sb[dt][:],
                        start=(dt == 0), stop=(dt == DT - 1),
                    )
                nc.vector.tensor_tensor(vt[t][:], acc[:], vbias_bc[:],
                                        mybir.AluOpType.add)

            # ---- q/k projections: qT/kT[ct][col, tok] (w stationary) ----
            for ct in range(8):
                wts = []
                for dt in range(DT):
                    wt = work.tile([128, 128], BF, name=f"w_{ct}_{dt}",
                                   tag="w", bufs=24)
                    nc.sync.dma_start(wt[:], wqk[dt, ct])
                    wts.append(wt)
                dest = qT[ct] if ct < 4 else kT[ct - 4]
                for tb in range(TB):
                    acc = psum.tile([128, 512], F32, name="acc_qk", tag="acc",
                                    bufs=2)
                    for dt in range(DT):
                        nc.tensor.matmul(
                            acc[:], wts[dt][:], xt_tiles[(dt, tb)][:],
                            start=(dt == 0), stop=(dt == DT - 1),
                        )
                    nc.scalar.activation(
                        dest[:, tb * 512:(tb + 1) * 512], acc[:],
                        mybir.ActivationFunctionType.Identity,
                        bias=bqk_sb[ct][:], scale=1.0,
                    )

            # ---- attention + chunked AllGather (qb descending), with the
            # out-projection of chunk qb+1 interleaved one AG behind so the
            # sync DMA queue never head-of-line blocks on a collective ----
            wout_sb = []
            for dt in range(DT):
                t = work.tile([128, CW], BF, name=f"wout{dt}", tag="p512",
                              bufs=17)
                nc.sync.dma_start(t[:], wout[dt * 128:(dt + 1) * 128, :])
                wout_sb.append(t)

            def attention_chunk(qb):
                nk = 4 * qb + 4
                for h in range(HLOC):
                    y_ps = psum.tile([128, 512], F32, name="y_ps", tag="y")
                    esum = work.tile([128, 1024], BF, name="esum", tag="esum",
                                     bufs=2)
                    # k-subtiles in pairs; PV/sum of pair pr-1 issue after
                    # QK/exp of pair pr so the PE never waits on a fresh exp
                    prev = None

                    def flush(prev_pair):
                        e, pr = prev_pair
                        for s_ in range(2):
                            kt = 2 * pr + s_
                            nc.tensor.matmul(
                                y_ps[:],
                                vt[kt][:, h * 128:(h + 1) * 128],
                                e[:, s_ * 512:(s_ + 1) * 512],
                                start=(kt == 0), stop=(kt == nk - 1),
                            )
                        if pr == 0:
                            nc.vector.tensor_copy(esum[:], e[:])
                        else:
                            nc.vector.tensor_tensor(esum[:], esum[:], e[:],
                                                    mybir.AluOpType.add)

                    for pr in range(nk // 2):
                        sc = psum.tile([128, 1024], F32, name="sc", tag="s",
                                       bufs=2)
                        for s_ in range(2):
                            kt = 2 * pr + s_
                            nc.tensor.matmul(
                                sc[:, s_ * 512:(s_ + 1) * 512],
                                kT[h][:, kt * 128:(kt + 1) * 128],
                                qT[h][:, qb * 512:(qb + 1) * 512],
                                start=True, stop=True,
                            )
                        e = work.tile([128, 1024], BF, name="expT", tag="expT",
                                      bufs=4)
                        nc.scalar.activation(
                            e[:], sc[:], mybir.ActivationFunctionType.Exp,
                            scale=SCALE,
                        )
                        pm = pr - (nk // 2 - 2)
                        if pm >= 0:
                            nc.vector.tensor_tensor(e[:], e[:],
                                                    pairmasks[pm][:],
                                                    mybir.AluOpType.mult)
                        if prev is not None:
                            flush(prev)
                        prev = (e, pr)
                    flush(prev)

                    esum_f = work.tile([128, 512], BF, name="esum_f",
                                       tag="esum_f", bufs=2)
                    nc.vector.tensor_tensor(esum_f[:], esum[:, 0:512],
                                            esum[:, 512:1024],
                                            mybir.AluOpType.add)
                    sum_ps = psum.tile([1, 512], F32, name="sum_ps", tag="y")
                    nc.tensor.matmul(sum_ps[:], ones[:], esum_f[:],
                                     start=True, stop=True)
                    recip = work.tile([1, 512], F32, name="recip", tag="recip",
                                      bufs=2)
                    nc.vector.reciprocal_approx_fast(recip[:], sum_ps[:])
                    rbc = work.tile([128, 512], F32, name="rbc", tag="rbc",
                                    bufs=2)
                    nc.gpsimd.partition_broadcast(rbc[:], recip[:], channels=128)
                    nc.vector.tensor_tensor(
                        yT[h][:, qb * 512:(qb + 1) * 512], y_ps[:], rbc[:],
                        mybir.AluOpType.mult,
                    )
                    nc.sync.dma_start(
                        ag_in[qb][h * 128:(h + 1) * 128, :],
                        yT[h][:, qb * 512:(qb + 1) * 512],
                    )
                nc.gpsimd.collective_compute(
                    "AllGather", mybir.AluOpType.bypass,
                    replica_groups=GROUPS,
                    ins=[ag_in[qb].ap()], outs=[ag_out[qb].ap()],
                )

            def outproj_chunk(qb):
                ygt = []
                for dt in range(DT):
                    t = work.tile([128, 512], BF, name=f"ygT_{qb}_{dt}",
                                  tag="xT", bufs=68)
                    nc.sync.dma_start(
                        t[:], ag_out[qb][dt * 128:(dt + 1) * 128, :])
                    ygt.append(t)
                for j in range(4):
                    acc = psum.tile([128, CW], F32, name="acc_o", tag="acc",
                                    bufs=2)
                    for dt in range(DT):
                        nc.tensor.matmul(
                            acc[:],
                            ygt[dt][:, j * 128:(j + 1) * 128],
                            wout_sb[dt][:],
                            start=(dt == 0), stop=(dt == DT - 1),
                        )
                    osb = work.tile([128, CW], F32, name="osb", tag="osb",
                                    bufs=3)
                    nc.vector.tensor_tensor(osb[:], acc[:], bias_bc[:],
                                            mybir.AluOpType.add)
                    tt = qb * 4 + j
                    nc.sync.dma_start(out[tt * 128:(tt + 1) * 128, :], osb[:])

            for qb in (3, 2, 1, 0):
                attention_chunk(qb)
            for qb in (3, 2, 1, 0):
                outproj_chunk(qb)

    nc.compile()
    return nc


def _prep_inputs(x, w_qkv, b_qkv, w_out, b_out):
    """Host-side sharding/layout. Returns in_maps for the 8 cores."""
    bf16 = ml_dtypes.bfloat16
    x = np.asarray(x, dtype=np.float32)
    w_qkv = np.asarray(w_qkv, dtype=np.float32)
    b_qkv = np.asarray(b_qkv, dtype=np.float32)
    w_out = np.asarray(w_out, dtype=np.float32)
    b_out = np.asarray(b_out, dtype=np.float32)

    xT_b = [np.ascontiguousarray(x[b].T).astype(bf16) for b in range(B)]

    in_maps = []
    for c in range(8):
        b, g = c // 4, c % 4
        cols = slice(CW * g, CW * (g + 1))
        wq = w_qkv[:, 0 * D:1 * D][:, cols]
        wk = w_qkv[:, 1 * D:2 * D][:, cols]
        wv_ = w_qkv[:, 2 * D:3 * D][:, cols]
        # (dt, ct, d, col) for q(0-3) then k(4-7), 128-col tiles
        wqk = np.concatenate([wq, wk], axis=1)            # [D, 1024]
        wqk = wqk.reshape(DT, 128, 8, 128).transpose(0, 2, 1, 3)
        wqk = np.ascontiguousarray(wqk).astype(bf16)
        wv_t = np.ascontiguousarray(wv_.reshape(DT, 128, CW)).astype(bf16)

        bq = b_qkv[0 * D:1 * D][cols]
        bk = b_qkv[1 * D:2 * D][cols]
        bv_ = b_qkv[2 * D:3 * D][cols]
        bqk = np.concatenate([bq, bk]).reshape(8, 128, 1).astype(np.float32)

        in_maps.append({
            "xT": xT_b[b],
            "wqk": wqk,
            "wv": wv_t,
            "bqk": np.ascontiguousarray(bqk),
            "bv": np.ascontiguousarray(bv_.reshape(1, CW)),
            "wout": np.ascontiguousarray(w_out[:, cols]).astype(bf16),
            "bout": np.ascontiguousarray(b_out[cols].reshape(1, CW)),
        })
    return in_maps


def kernel(x, w_qkv, b_qkv, w_out, b_out, _trace=False, _trace_kwargs=None):
    from concourse.bass_utils import run_bass_kernel_spmd

    if "nc" not in _cache:
        _cache["nc"] = _build()
    nc = _cache["nc"]

    in_maps = _prep_inputs(x, w_qkv, b_qkv, w_out, b_out)
    res = run_bass_kernel_spmd(
        nc, in_maps, core_ids=list(range(8)),
        trace=_trace, **(_trace_kwargs or {}),
    )

    out = np.empty((B, S, D), dtype=np.float32)
    for c in range(8):
        b, g = c // 4, c % 4
        out[b][:, CW * g:CW * (g + 1)] = res.results[c]["out"]
    kernel.last_result = res
    return out
